# revision 6
# baseline (speedup 1.0000x reference)
"""Trainium2 Bass kernel: 2-layer shared-weight LSTM with residual.

x:[1024,200,128], W/U:[128,512], b:[512]; two stacked LSTM layers sharing
(W,U,b); layer 2 residual (y = h2raw + h1); seq_len ignored (reference runs
full T).  Data-parallel over batch: 8 cores x 128 rows.

Per-core architecture ("S2"): the 128 batch rows are split into TWO
independent streams of 64 that software-pipeline through the engines,
hiding the serial per-step latency chain.  Features/gates live on SBUF
partitions, batch on the free axis.

Each stream keeps a 32-slot ring; slot u = [h2raw(u-1) | h1(u) | x(u+1)]
(64 cols each).  Fused unit u computes L2 step u-1 and L1 step u:
  z2(u-1) = W h1(u-1) + U (h2raw(u-2)+h1(u-2)) + b   (residual split into
  z1(u)   = W x(u)    + U h1(u-1)              + b    two matmuls)
Gate quarters in one PSUM bank [128, 4*128]: quarter q = [L2 64 | L1 64].
Matmuls per stream-unit (bf16):
  - bias opener: rank-4 matmul bias_lhsT[4,128] x quarter_masks[4,512]
    (injects per-gate bias into PSUM; start=True)
  - U x slot(u-1)[0:128]   = [h2raw(u-2)|h1(u-1)] -> [L2|L1]  (N=128) x4
  - W x slot(u-1)[64:192]  = [h1(u-1)|x(u)]       -> [L2|L1]  (N=128) x4
  - U x slot(u-2)[64:128]  = h1(u-2)              -> L2 only  (N=64)  x4
One SIGMOID activation covers all four gates [128,512]: the g gate uses
tanh(z) = 2*sigmoid(2z)-1 with W_g,U_g,b_g host-scaled by 2, fixed up in
the DVE c-path:  c' = f*c + (2*sig_g - 1)*i = (f*c - i) + (2*sig_g)*i.
One merged TANH covers both layers' c.  y = h2raw + h1 is staged by GpSimd
off the critical chain and DMA'd out 8 steps per transfer.
"""

import numpy as np
import ml_dtypes

import concourse.bass as bass
import concourse.tile as tile
from concourse import bacc, mybir
from concourse.bass_utils import run_bass_kernel_spmd

B, T, D = 1024, 200, 128
NCORES = 8
BL = B // NCORES          # 128 batch rows per core
NS = 2                    # streams per core
SW = BL // NS             # 64 batch rows per stream
RD = 32                   # h-ring depth (slots)
G8 = T // 8               # 25 groups of 8 timesteps for x/y DMA

F32 = mybir.dt.float32
F32R = mybir.dt.float32r
BF16 = mybir.dt.bfloat16

SIG = mybir.ActivationFunctionType.Sigmoid
TANH = mybir.ActivationFunctionType.Tanh

# gate quarter order in W/U/b and in the PSUM bank: i, f, g, o (Keras)
QI, QF, QG, QO = 0, 1, 2, 3


def _build(nc):
    # x0: step 0, [D, 128] (both streams).  xg: steps 8g+1..8g+8, the
    # entry for t=200 is zero-padded (read by unit 200, never used).
    x0_d = nc.dram_tensor("x0", [D, BL], BF16, kind="ExternalInput")
    xg_d = nc.dram_tensor("xg", [G8, D, 8, NS, SW], BF16, kind="ExternalInput")
    w_d = nc.dram_tensor("w", [D, 4 * D], BF16, kind="ExternalInput")
    u_d = nc.dram_tensor("u", [D, 4 * D], BF16, kind="ExternalInput")
    bl_d = nc.dram_tensor("bl", [4, D], BF16, kind="ExternalInput")
    mk_d = nc.dram_tensor("mk", [4, 4 * D], BF16, kind="ExternalInput")
    y_d = nc.dram_tensor("y", [G8, D, 8, NS, SW], BF16, kind="ExternalOutput")

    with tile.TileContext(nc) as tc:
        with (
            tc.tile_pool(name="singles", bufs=1) as singles,
            tc.tile_pool(name="psum", bufs=2, space="PSUM") as pspool,
            tc.tile_pool(name="gates", bufs=3) as gpool,
            tc.tile_pool(name="tmp", bufs=2) as tpool,
        ):
            w_sb = singles.tile([D, 4 * D], BF16)
            u_sb = singles.tile([D, 4 * D], BF16)
            bl_sb = singles.tile([4, D], BF16)
            mk_sb = singles.tile([4, 4 * D], BF16)
            nc.sync.dma_start(w_sb[:], w_d[:])
            nc.sync.dma_start(u_sb[:], u_d[:])
            nc.sync.dma_start(bl_sb[:], bl_d[:])
            nc.sync.dma_start(mk_sb[:], mk_d[:])

            # per-stream state
            ring = [singles.tile([D, RD, 3 * SW], BF16, name=f"ring{s}")
                    for s in range(NS)]
            yst = [singles.tile([D, 16, SW], BF16, name=f"yst{s}")
                   for s in range(NS)]
            c_st = [singles.tile([D, 2 * SW], BF16, name=f"cst{s}")
                    for s in range(NS)]
            for s in range(NS):
                nc.vector.memset(ring[s][:], 0.0)
                nc.vector.memset(c_st[s][:], 0.0)
                # x(0) -> slot 31 cols 128:192
                nc.sync.dma_start(ring[s][:, (RD - 1), 2 * SW:3 * SW],
                                  x0_d[:, s * SW:(s + 1) * SW])

            def wq(q):
                return w_sb[:, q * D:(q + 1) * D]

            def uq(q):
                return u_sb[:, q * D:(q + 1) * D]

            def xdma(g):
                # x(8g+1..8g+8) -> slots (8g..8g+7)%RD cols 128:192
                base = (8 * g) % RD
                for s in range(NS):
                    nc.sync.dma_start(
                        ring[s][:, base:base + 8, 2 * SW:3 * SW],
                        xg_d[g, :, :, s, :])

            def ydma(g):
                base = (8 * g) % 16
                for s in range(NS):
                    nc.sync.dma_start(
                        y_d[g, :, :, s, :],
                        yst[s][:, base:base + 8, :])

            xdma(0)
            xdma(1)

            for u in range(T + 1):
                if u % 8 == 0 and 2 <= u // 8 + 1 <= G8 - 1:
                    xdma(u // 8 + 1)
                for s in range(NS):
                    sl_u = ring[s][:, u % RD, :]
                    sl_1 = ring[s][:, (u - 1) % RD, :]
                    sl_2 = ring[s][:, (u - 2) % RD, :]

                    ps = pspool.tile([D, 4 * 2 * SW], F32, tag=f"ps{s}",
                                     name=f"ps{s}")
                    # bias opener: rank-4, resets the PSUM bank
                    nc.tensor.matmul(ps[:], bl_sb[:], mk_sb[:],
                                     start=True, stop=False)
                    for q in range(4):
                        qc = ps[:, q * 2 * SW:(q + 1) * 2 * SW]
                        # U (h2raw(u-2)+h1(u-2)) residual completion: L2 only
                        nc.tensor.matmul(qc[:, 0:SW], uq(q),
                                         sl_2[:, SW:2 * SW],
                                         start=False, stop=False)
                        # W [h1(u-1) | x(u)] -> [L2|L1]
                        nc.tensor.matmul(qc[:], wq(q), sl_1[:, SW:3 * SW],
                                         start=False, stop=False)
                        # U [h2raw(u-2) | h1(u-1)] -> [L2|L1]
                        nc.tensor.matmul(qc[:], uq(q), sl_1[:, 0:2 * SW],
                                         start=False, stop=(q == 3))

                    gt = gpool.tile([D, 4 * 2 * SW], BF16, tag=f"gt{s}",
                                    name=f"gt{s}")
                    nc.scalar.activation(gt[:], ps[:], SIG)
                    gi = gt[:, QI * 2 * SW:(QI + 1) * 2 * SW]
                    gf = gt[:, QF * 2 * SW:(QF + 1) * 2 * SW]
                    gg = gt[:, QG * 2 * SW:(QG + 1) * 2 * SW]
                    go = gt[:, QO * 2 * SW:(QO + 1) * 2 * SW]

                    m1 = tpool.tile([D, 2 * SW], BF16, tag=f"m1{s}",
                                    name=f"m1{s}")
                    pp = tpool.tile([D, 2 * SW], BF16, tag=f"pp{s}",
                                    name=f"pp{s}")
                    rr = tpool.tile([D, 2 * SW], BF16, tag=f"rr{s}",
                                    name=f"rr{s}")
                    tcn = tpool.tile([D, 2 * SW], BF16, tag=f"tc{s}",
                                     name=f"tc{s}")
                    # c' = (f*c - i) + (2*sig_g)*i ; g quarter holds
                    # sigmoid(2 z_g) via host-scaled weights
                    nc.vector.tensor_mul(m1[:], gf[:], c_st[s][:])
                    nc.vector.scalar_tensor_tensor(
                        pp[:], gg[:], 2.0, gi[:],
                        mybir.AluOpType.mult, mybir.AluOpType.mult)
                    nc.vector.tensor_sub(rr[:], m1[:], gi[:])
                    if u == 0:
                        # only the L1 half may update c (keep c2 = 0 for
                        # the real L2 step 0 in unit 1)
                        nc.vector.tensor_add(c_st[s][:, SW:2 * SW],
                                             rr[:, SW:2 * SW],
                                             pp[:, SW:2 * SW])
                    else:
                        nc.vector.tensor_add(c_st[s][:], rr[:], pp[:])
                    nc.scalar.activation(tcn[:], c_st[s][:], TANH)
                    # [h2raw(u-1) | h1(u)] -> ring slot u
                    nc.vector.tensor_mul(sl_u[:, 0:2 * SW], go[:], tcn[:])
                    if u >= 1:
                        # y(u-1) = h2raw(u-1) + h1(u-1), staged off-chain
                        nc.gpsimd.tensor_add(
                            yst[s][:, (u - 1) % 16, :],
                            sl_u[:, 0:SW], sl_1[:, SW:2 * SW])
                if u % 8 == 0 and u >= 8:
                    ydma(u // 8 - 1)

    nc.finalize()
    return nc


_CACHED = {}


def _get_nc():
    if "nc" not in _CACHED:
        nc = bacc.Bacc("TRN2", target_bir_lowering=False, debug=False,
                       num_devices=NCORES)
        _CACHED["nc"] = _build(nc)
    return _CACHED["nc"]


def kernel(x, W, U, b, seq_len):
    assert x.shape == (B, T, D)
    nc = _get_nc()
    bf = ml_dtypes.bfloat16

    Wf = np.asarray(W, dtype=np.float32).copy()
    Uf = np.asarray(U, dtype=np.float32).copy()
    bfv = np.asarray(b, dtype=np.float32).copy()
    # g gate via sigmoid: tanh(z) = 2*sigmoid(2z)-1 -> scale W_g,U_g,b_g by 2
    Wf[:, 2 * D:3 * D] *= 2.0
    Uf[:, 2 * D:3 * D] *= 2.0
    bfv[2 * D:3 * D] *= 2.0
    Wc = np.ascontiguousarray(Wf.astype(bf))
    Uc = np.ascontiguousarray(Uf.astype(bf))
    blc = np.ascontiguousarray(bfv.reshape(4, D).astype(bf))  # [4, D]
    mk = np.zeros((4, 4 * D), dtype=np.float32)
    for q in range(4):
        mk[q, q * D:(q + 1) * D] = 1.0
    mk = np.ascontiguousarray(mk.astype(bf))

    in_maps = []
    for c in range(NCORES):
        xc = (np.asarray(x[c * BL:(c + 1) * BL], dtype=np.float32)
              .transpose(1, 2, 0))                         # [T, D, BL] f32
        x0 = np.ascontiguousarray(xc[0].astype(bf))        # [D, BL]
        # groups of 8: x(8g+1..8g+8), pad t=200 with zeros
        xs = np.zeros((T + 8, D, BL), dtype=np.float32)
        xs[:T - 1] = xc[1:]
        xg = (xs[:8 * G8].reshape(G8, 8, D, NS, SW)
              .transpose(0, 2, 1, 3, 4))                   # [G8, D, 8, 2, SW]
        xg = np.ascontiguousarray(xg.astype(bf))
        in_maps.append({"x0": x0, "xg": xg, "w": Wc, "u": Uc,
                        "bl": blc, "mk": mk})

    res = run_bass_kernel_spmd(nc, in_maps, core_ids=list(range(NCORES)))

    y = np.empty((B, T, D), dtype=np.float32)
    for c in range(NCORES):
        yg = res.results[c]["y"].astype(np.float32)        # [G8, D, 8, 2, SW]
        yt = yg.transpose(0, 2, 3, 4, 1).reshape(T, BL, D)  # [T, BL, D]
        y[c * BL:(c + 1) * BL] = yt.transpose(1, 0, 2)
    return y


# revision 12
# speedup vs baseline: 1.0707x; 1.0707x over previous
"""Trainium2 Bass kernel: 2-layer shared-weight LSTM with residual.

x:[1024,200,128], W/U:[128,512], b:[512]; two stacked LSTM layers sharing
(W,U,b); layer 2 residual (y = h2raw + h1); seq_len ignored (reference runs
full T).  Data-parallel over batch: 8 cores x 128 rows.

Per-core architecture ("S2"): the 128 batch rows are split into TWO
independent streams of 64 that software-pipeline through the engines,
hiding the serial per-step latency chain.  Features/gates live on SBUF
partitions, batch on the free axis.

Each stream keeps a 32-slot ring; slot u = [h2raw(u-1) | h1(u) | x(u+1)]
(64 cols each).  Fused unit u computes L2 step u-1 and L1 step u:
  z2(u-1) = W h1(u-1) + U (h2raw(u-2)+h1(u-2)) + b   (residual split into
  z1(u)   = W x(u)    + U h1(u-1)              + b    two matmuls)
Gate quarters in one PSUM bank [128, 4*128]: quarter q = [L2 64 | L1 64].
Matmuls per stream-unit (bf16):
  - bias opener: rank-4 matmul bias_lhsT[4,128] x quarter_masks[4,512]
    (injects per-gate bias into PSUM; start=True)
  - U x slot(u-1)[0:128]   = [h2raw(u-2)|h1(u-1)] -> [L2|L1]  (N=128) x4
  - W x slot(u-1)[64:192]  = [h1(u-1)|x(u)]       -> [L2|L1]  (N=128) x4
  - U x slot(u-2)[64:128]  = h1(u-2)              -> L2 only  (N=64)  x4
One SIGMOID activation covers all four gates [128,512]: the g gate uses
tanh(z) = 2*sigmoid(2z)-1 with W_g,U_g,b_g host-scaled by 2, fixed up in
the DVE c-path:  c' = f*c + (2*sig_g - 1)*i = (f*c - i) + (2*sig_g)*i.
One merged TANH covers both layers' c.  y = h2raw + h1 is staged by GpSimd
off the critical chain and DMA'd out 8 steps per transfer.
"""

import numpy as np
import ml_dtypes

import concourse.bass as bass
import concourse.tile as tile
from concourse import bacc, mybir
from concourse.bass_utils import run_bass_kernel_spmd

B, T, D = 1024, 200, 128
NCORES = 8
BL = B // NCORES          # 128 batch rows per core
NS = 2                    # streams per core
SW = BL // NS             # 64 batch rows per stream
RD = 32                   # h-ring depth (slots)
G8 = T // 8               # 25 groups of 8 timesteps for x/y DMA

F32 = mybir.dt.float32
F32R = mybir.dt.float32r
BF16 = mybir.dt.bfloat16

import os
NDUMMY = int(os.environ.get("K_DUMMY", "1"))   # dummy matmuls per stream-unit
DN = 128                                       # dummy matmul free size

SIG = mybir.ActivationFunctionType.Sigmoid
TANH = mybir.ActivationFunctionType.Tanh

# gate quarter order in W/U/b and in the PSUM bank: i, f, g, o (Keras)
QI, QF, QG, QO = 0, 1, 2, 3


def _build(nc):
    # x0: step 0, [D, 128] (both streams).  xg: steps 8g+1..8g+8, the
    # entry for t=200 is zero-padded (read by unit 200, never used).
    x0_d = nc.dram_tensor("x0", [D, BL], BF16, kind="ExternalInput")
    xg_d = nc.dram_tensor("xg", [G8, D, 8, NS, SW], BF16, kind="ExternalInput")
    w_d = nc.dram_tensor("w", [D, 4 * D], BF16, kind="ExternalInput")
    u_d = nc.dram_tensor("u", [D, 4 * D], BF16, kind="ExternalInput")
    bl_d = nc.dram_tensor("bl", [4, D], BF16, kind="ExternalInput")
    mk_d = nc.dram_tensor("mk", [4, 4 * D], BF16, kind="ExternalInput")
    y_d = nc.dram_tensor("y", [G8, D, 8, NS, SW], BF16, kind="ExternalOutput")

    with tile.TileContext(nc) as tc:
        with (
            tc.tile_pool(name="singles", bufs=1) as singles,
            tc.tile_pool(name="psum", bufs=2, space="PSUM") as pspool,
            tc.tile_pool(name="gates", bufs=3) as gpool,
            tc.tile_pool(name="tmp", bufs=2) as tpool,
        ):
            w_sb = singles.tile([D, 4 * D], BF16)
            u_sb = singles.tile([D, 4 * D], BF16)
            bl_sb = singles.tile([4, D], BF16)
            mk_sb = singles.tile([4, 4 * D], BF16)
            nc.sync.dma_start(w_sb[:], w_d[:])
            nc.sync.dma_start(u_sb[:], u_d[:])
            nc.sync.dma_start(bl_sb[:], bl_d[:])
            nc.sync.dma_start(mk_sb[:], mk_d[:])

            # per-stream state; ring slot u = [h2n(u-1) | h1(u) | x(u+1)]
            ring = [singles.tile([D, RD, 3 * SW], BF16, name=f"ring{s}")
                    for s in range(NS)]
            c_st = [singles.tile([D, 2 * SW], BF16, name=f"cst{s}")
                    for s in range(NS)]
            for s in range(NS):
                nc.vector.memset(ring[s][:], 0.0)
                nc.vector.memset(c_st[s][:], 0.0)
                # x(0) -> slot 31 cols 128:192
                nc.sync.dma_start(ring[s][:, (RD - 1), 2 * SW:3 * SW],
                                  x0_d[:, s * SW:(s + 1) * SW])

            def wq(q):
                return w_sb[:, q * D:(q + 1) * D]

            def uq(q):
                return u_sb[:, q * D:(q + 1) * D]

            def xdma(g):
                # x(8g+1..8g+8) -> slots (8g..8g+7)%RD cols 128:192
                base = (8 * g) % RD
                for s in range(NS):
                    nc.sync.dma_start(
                        ring[s][:, base:base + 8, 2 * SW:3 * SW],
                        xg_d[g, :, :, s, :])

            def ydma(g):
                # y(8g+j) = h2n(8g+j) = ring slot (8g+j+1) cols 0:SW
                s0 = (8 * g + 1) % RD
                for s in range(NS):
                    if s0 + 8 <= RD:
                        nc.sync.dma_start(y_d[g, :, :, s, :],
                                          ring[s][:, s0:s0 + 8, 0:SW])
                    else:
                        n1 = RD - s0
                        nc.sync.dma_start(y_d[g, :, 0:n1, s, :],
                                          ring[s][:, s0:RD, 0:SW])
                        nc.sync.dma_start(y_d[g, :, n1:8, s, :],
                                          ring[s][:, 0:8 - n1, 0:SW])

            xdma(0)
            xdma(1)

            for u in range(T + 1):
                if u % 8 == 0 and 2 <= u // 8 + 1 <= G8 - 1:
                    xdma(u // 8 + 1)
                for s in range(NS):
                    sl_u = ring[s][:, u % RD, :]
                    sl_1 = ring[s][:, (u - 1) % RD, :]

                    ps = pspool.tile([D, 4 * 2 * SW], F32, tag=f"ps{s}",
                                     name=f"ps{s}")
                    # bias opener: rank-4, resets the PSUM bank
                    nc.tensor.matmul(ps[:], bl_sb[:], mk_sb[:],
                                     start=True, stop=False)
                    for q in range(4):
                        qc = ps[:, q * 2 * SW:(q + 1) * 2 * SW]
                        # W [h1(u-1) | x(u)] -> [L2|L1]
                        nc.tensor.matmul(qc[:], wq(q), sl_1[:, SW:3 * SW],
                                         start=False, stop=False)
                    for q in range(4):
                        qc = ps[:, q * 2 * SW:(q + 1) * 2 * SW]
                        # U [h2n(u-2) | h1(u-1)] -> [L2|L1]
                        nc.tensor.matmul(qc[:], uq(q), sl_1[:, 0:2 * SW],
                                         start=False, stop=(q == 3))

                    gt = gpool.tile([D, 4 * 2 * SW], BF16, tag=f"gt{s}",
                                    name=f"gt{s}")
                    nc.scalar.activation(gt[:], ps[:], SIG)
                    gi = gt[:, QI * 2 * SW:(QI + 1) * 2 * SW]
                    gf = gt[:, QF * 2 * SW:(QF + 1) * 2 * SW]
                    gg = gt[:, QG * 2 * SW:(QG + 1) * 2 * SW]
                    go = gt[:, QO * 2 * SW:(QO + 1) * 2 * SW]

                    m1 = tpool.tile([D, 2 * SW], BF16, tag=f"m1{s}",
                                    name=f"m1{s}")
                    pp = tpool.tile([D, 2 * SW], BF16, tag=f"pp{s}",
                                    name=f"pp{s}")
                    rr = tpool.tile([D, 2 * SW], BF16, tag=f"rr{s}",
                                    name=f"rr{s}")
                    tcn = tpool.tile([D, 2 * SW], BF16, tag=f"tc{s}",
                                     name=f"tc{s}")
                    # c' = (f*c - i) + (2*sig_g)*i ; g quarter holds
                    # sigmoid(2 z_g) via host-scaled weights
                    nc.vector.tensor_mul(m1[:], gf[:], c_st[s][:])
                    nc.vector.scalar_tensor_tensor(
                        pp[:], gg[:], 2.0, gi[:],
                        mybir.AluOpType.mult, mybir.AluOpType.mult)
                    nc.vector.tensor_sub(rr[:], m1[:], gi[:])
                    if u == 0:
                        # only the L1 half may update c (keep c2 = 0 for
                        # the real L2 step 0 in unit 1)
                        nc.vector.tensor_add(c_st[s][:, SW:2 * SW],
                                             rr[:, SW:2 * SW],
                                             pp[:, SW:2 * SW])
                    else:
                        nc.vector.tensor_add(c_st[s][:], rr[:], pp[:])
                    nc.scalar.activation(tcn[:], c_st[s][:], TANH)
                    # [h2raw(u-1) | h1(u)] -> ring slot u
                    nc.vector.tensor_mul(sl_u[:, 0:2 * SW], go[:], tcn[:])
                    if u >= 1:
                        # residual: slot cols 0:SW become h2n(u-1) = y(u-1)
                        nc.vector.tensor_add(sl_u[:, 0:SW], sl_u[:, 0:SW],
                                             sl_1[:, SW:2 * SW])
                    # dummy matmuls: keep the PE array busy through the
                    # pointwise tail so the p-state stays at full clock
                    for _ in range(NDUMMY):
                        dmy = pspool.tile([D, DN], F32, tag="dmy",
                                          name="dmy")
                        nc.tensor.matmul(dmy[:], w_sb[:, 0:D],
                                         u_sb[:, 0:DN], start=True,
                                         stop=True)
                if u % 8 == 0 and u >= 8:
                    ydma(u // 8 - 1)

    nc.finalize()
    return nc


_CACHED = {}


def _get_nc():
    if "nc" not in _CACHED:
        nc = bacc.Bacc("TRN2", target_bir_lowering=False, debug=False,
                       num_devices=NCORES)
        _CACHED["nc"] = _build(nc)
    return _CACHED["nc"]


def kernel(x, W, U, b, seq_len):
    assert x.shape == (B, T, D)
    nc = _get_nc()
    bf = ml_dtypes.bfloat16

    Wf = np.asarray(W, dtype=np.float32).copy()
    Uf = np.asarray(U, dtype=np.float32).copy()
    bfv = np.asarray(b, dtype=np.float32).copy()
    # g gate via sigmoid: tanh(z) = 2*sigmoid(2z)-1 -> scale W_g,U_g,b_g by 2
    Wf[:, 2 * D:3 * D] *= 2.0
    Uf[:, 2 * D:3 * D] *= 2.0
    bfv[2 * D:3 * D] *= 2.0
    Wc = np.ascontiguousarray(Wf.astype(bf))
    Uc = np.ascontiguousarray(Uf.astype(bf))
    blc = np.ascontiguousarray(bfv.reshape(4, D).astype(bf))  # [4, D]
    mk = np.zeros((4, 4 * D), dtype=np.float32)
    for q in range(4):
        mk[q, q * D:(q + 1) * D] = 1.0
    mk = np.ascontiguousarray(mk.astype(bf))

    in_maps = []
    for c in range(NCORES):
        xc = (np.asarray(x[c * BL:(c + 1) * BL], dtype=np.float32)
              .transpose(1, 2, 0))                         # [T, D, BL] f32
        x0 = np.ascontiguousarray(xc[0].astype(bf))        # [D, BL]
        # groups of 8: x(8g+1..8g+8), pad t=200 with zeros
        xs = np.zeros((T + 8, D, BL), dtype=np.float32)
        xs[:T - 1] = xc[1:]
        xg = (xs[:8 * G8].reshape(G8, 8, D, NS, SW)
              .transpose(0, 2, 1, 3, 4))                   # [G8, D, 8, 2, SW]
        xg = np.ascontiguousarray(xg.astype(bf))
        in_maps.append({"x0": x0, "xg": xg, "w": Wc, "u": Uc,
                        "bl": blc, "mk": mk})

    res = run_bass_kernel_spmd(nc, in_maps, core_ids=list(range(NCORES)))

    y = np.empty((B, T, D), dtype=np.float32)
    for c in range(NCORES):
        yg = res.results[c]["y"].astype(np.float32)        # [G8, D, 8, 2, SW]
        yt = yg.transpose(0, 2, 3, 4, 1).reshape(T, BL, D)  # [T, BL, D]
        y[c * BL:(c + 1) * BL] = yt.transpose(1, 0, 2)
    return y
